# revision 29
# baseline (speedup 1.0000x reference)
"""Trainium2 Bass kernel for the gated-attention nn.Module.

Math (per batch element b):
    deg   = rel_pos.sum(-1)                        # [N]
    gate  = sigmoid(deg * W_d + b_d)               # [N, D]
    xg    = x * gate
    qkv   = xg @ W_qkv.T + b_qkv                   # [N, 3D]
    qk, value, res = split(qkv); qk = sigmoid(qk)
    attn  = (qk @ qk.T) * scale * rel_pos          # [N, N]
    attn  = attn / (attn.sum(-1, keepdims) + 1e-6)
    out   = relu(attn @ value + res)               # [N, D]

Sharding: pure data-parallel over batch, B == 8 == n_cores, one batch
element per NeuronCore, no collectives.

Per-core schedule (two phases):
  phase A: stream rel_pos f32 on the sync HWDGE queue at full rate and
      keep it resident in f32 (no cast).  Row sums (deg) are computed on
      the DVE for even tiles and on the scalar engine (activation
      accumulator, scratch output) for odd tiles to balance load.
      gate/xg, the XBAR transposes of xg and the qkv projections
      pipeline in the stream's shadow.  qk is written as sigmoid(..) in
      fp8e4 packed [128, 2, N] layout (k-halves on dim 1) for DoubleRow
      matmuls, with its bias folded into the sigmoid activation.
      value is stored fp8 and rearranged into DoubleRow [128, 2, D]
      blocks (partition p holds rows 2p/2p+1 of a 256-row block) with
      parity DMAs on the otherwise idle SWDGE queue.
  phase B: per query row-tile, one DoubleRow fp8 matmul sweep produces
      the full 2048-wide score row in PSUM (K=256 per instruction), a
      single full-row DVE scalar_tensor_tensor fuses the scale+rel_pos
      multiply, the fp8 cast and the row-sum accumulation.  The fp8 row
      is XBAR-transposed as PAIRS via a bf16 bitcast view: the resulting
      layout PT[p, 2q+h] = P[q, 256*j2 + 2p + h] is exactly DoubleRow
      lhsT layout, so attn @ value also runs as 8 DoubleRow matmuls per
      tile (K=256 each) against the interleaved value blocks.  Row
      normalization is applied after the matmul as a per-partition scale
      fused with the residual add.
"""

import math
from contextlib import ExitStack

import numpy as np

import concourse.bass as bass
import concourse.tile as tile
from concourse import bacc, mybir
from concourse.bass import ts
from concourse.bass_utils import run_bass_kernel_spmd
from concourse.masks import make_identity

B, N, D = 8, 2048, 256
E = 3 * D  # 768
NT = N // 128  # 16 row tiles
NB = N // 256  # 8 DoubleRow blocks
DC = D // 128  # 2 dim chunks
SCALE = 1.0 / math.sqrt(32.0)
EPS = 1e-6

F32 = mybir.dt.float32
BF16 = mybir.dt.bfloat16
FP8 = mybir.dt.float8e4

AL = mybir.AluOpType
AF = mybir.ActivationFunctionType
DR = mybir.MatmulPerfMode.DoubleRow
DRSW = mybir.MatmulPerfMode.DoubleRowSwInterleave


def build_kernel(ctx: ExitStack, tc: tile.TileContext, io: dict):
    nc = tc.nc
    x_d = io["x"]          # [N, D]   f32
    rel_d = io["rel_pos"]  # [N, N]   f32
    wq_d = io["W_qkv"]     # [E, D]   f32
    bq_d = io["b_qkv"]     # [E]      f32
    wd_d = io["W_d"]       # [D, 1]   f32
    bd_d = io["b_d"]       # [D]      f32
    out_d = io["out"]      # [N, D]   f32

    # ---------------- pools ----------------
    consts = ctx.enter_context(tc.tile_pool(name="consts", bufs=1))
    resid = ctx.enter_context(tc.tile_pool(name="resid", bufs=1))
    xbuf = ctx.enter_context(tc.tile_pool(name="xbuf", bufs=3))
    ptbuf = ctx.enter_context(tc.tile_pool(name="ptbuf", bufs=3))
    opool = ctx.enter_context(tc.tile_pool(name="opool", bufs=2))
    small = ctx.enter_context(tc.tile_pool(name="small", bufs=8))
    sc_ps = ctx.enter_context(tc.tile_pool(name="sc_ps", bufs=2, space="PSUM"))
    mm_ps = ctx.enter_context(tc.tile_pool(name="mm_ps", bufs=2, space="PSUM"))
    pt_ps = ctx.enter_context(tc.tile_pool(name="pt_ps", bufs=2, space="PSUM"))

    # ---------------- constants ----------------
    ident = consts.tile([128, 128], BF16)
    make_identity(nc, ident)

    wd_bc = consts.tile([128, D], F32)
    nc.sync.dma_start(
        out=wd_bc,
        in_=bass.AP(tensor=wd_d.tensor, offset=wd_d.offset, ap=[[0, 128], [1, D]]),
    )
    bd_bc = consts.tile([128, D], F32)
    nc.sync.dma_start(
        out=bd_bc,
        in_=bass.AP(tensor=bd_d.tensor, offset=bd_d.offset, ap=[[0, 128], [1, D]]),
    )

    ones_row = consts.tile([1, 512], BF16)
    nc.vector.memset(ones_row, 1.0)
    bq_row_f = consts.tile([1, E], F32)
    nc.sync.dma_start(
        out=bq_row_f,
        in_=bass.AP(tensor=bq_d.tensor, offset=bq_d.offset, ap=[[1, 1], [1, E]]),
    )
    bq_row = consts.tile([1, E], BF16)
    nc.vector.tensor_copy(out=bq_row, in_=bq_row_f)
    bq_col = consts.tile([128, DC], F32)
    nc.sync.dma_start(
        out=bq_col,
        in_=bass.AP(tensor=bq_d.tensor, offset=bq_d.offset, ap=[[1, 128], [128, DC]]),
    )

    # W_qkv loaded with SWDGE bf16 cast -> PE-transposed WqT[dc] = W_qkv.T
    wq_nat_bf = consts.tile([128, 6, D], BF16)
    nc.gpsimd.dma_start(out=wq_nat_bf, in_=wq_d.rearrange("(c p) d -> p c d", p=128))
    wqT = [consts.tile([128, E], BF16, tag=f"wqT{dc}", name=f"wqT{dc}") for dc in range(DC)]
    for c in range(6):
        for dc in range(DC):
            pt = pt_ps.tile([128, 128], BF16, tag="pt", name="pt_w", padded_shape=[128, 1024])
            nc.tensor.transpose(pt, wq_nat_bf[:, c, ts(dc, 128)], ident)
            nc.scalar.copy(out=wqT[dc][:, ts(c, 128)], in_=pt)

    # ---------------- resident tensors ----------------
    relf = [resid.tile([128, N], FP8, tag=f"relf{j}", name=f"relf{j}") for j in range(NT)]
    qkP = resid.tile([128, 2, N], FP8)
    xgT = resid.tile([128, 2, N], BF16)
    vp = [resid.tile([128, D], BF16, tag=f"vp{j}", name=f"vp{j}") for j in range(NT)]
    res = [resid.tile([128, D], F32, tag=f"res{j}", name=f"res{j}") for j in range(NT)]
    deg = resid.tile([128, NT], F32)
    gates = [resid.tile([128, D], F32, tag=f"gate{j}", name=f"gate{j}") for j in range(NT)]
    # attention score rows, built in column halves as qk chunks land
    P = [resid.tile([128, N], BF16, tag=f"P{j}", name=f"P{j}") for j in range(NT)]
    zcs = resid.tile([128, 2 * NT], F32)

    # x preloaded on the SAME in-order SWDGE queue, AHEAD of the rel
    # stream: guarantees x has landed before the stream monopolizes HBM
    xts = [resid.tile([128, D], F32, tag=f"xt{j}", name=f"xt{j}") for j in range(NT)]
    for i in range(NT):
        nc.gpsimd.dma_start(out=xts[i], in_=x_d[ts(i, 128), :])
    for i in range(NT):
        nc.gpsimd.dma_start(out=relf[i], in_=rel_d[ts(i, 128), :])

    # ---------------- phase A stages ----------------
    def a_deg(i):
        # deg on the scalar engine (activation accumulator, scratch out):
        # DVE is fully booked with gate/xg and the score STTs
        degscr = ptbuf.tile([128, N], BF16, tag="PT", name="degscr")
        nc.scalar.activation(
            out=degscr[:, 0:N].bitcast(FP8)[:, 0:N],
            in_=relf[i],
            func=AF.Copy,
            accum_out=deg[:, i : i + 1],
        )

    def a_gate(i):
        nc.vector.scalar_tensor_tensor(
            out=gates[i],
            in0=wd_bc,
            scalar=deg[:, i : i + 1],
            in1=bd_bc,
            op0=AL.mult,
            op1=AL.add,
        )
        nc.scalar.activation(out=gates[i], in_=gates[i], func=AF.Sigmoid)

    def a_xg(i):
        xg = xbuf.tile([128, D], BF16, tag="xg", name="xg", bufs=6)
        nc.vector.tensor_tensor(out=xg, in0=xts[i], in1=gates[i], op=AL.mult)
        for dc in range(DC):
            pt = pt_ps.tile([128, 128], BF16, tag="pt", name="pt_xg", padded_shape=[128, 1024])
            nc.tensor.transpose(pt, xg[:, ts(dc, 128)], ident)
            nc.scalar.copy(out=xgT[:, dc, ts(i, 128)], in_=pt)

    def a_qkv(i):
        pv = mm_ps.tile([128, 512], F32, tag="mm", name="pv")
        for dc in range(DC):
            nc.tensor.matmul(
                pv,
                lhsT=xgT[:, dc, ts(i, 128)],
                rhs=wqT[dc][:, D : 3 * D],
                start=(dc == 0),
                stop=False,
            )
        nc.tensor.matmul(
            pv, lhsT=ones_row[:, 0:128], rhs=bq_row[:, D : 3 * D], start=False, stop=True
        )
        nc.scalar.copy(out=vp[i], in_=pv[:, 0:D])
        nc.scalar.copy(out=res[i], in_=pv[:, D : 2 * D])
        if i % 4 == 3:
            g = i // 4
            for h in range(DC):
                pq = mm_ps.tile([128, 512], F32, tag="mm", name="pq")
                for dc in range(DC):
                    nc.tensor.matmul(
                        pq,
                        lhsT=wqT[dc][:, ts(h, 128)],
                        rhs=xgT[:, dc, ts(g, 512)],
                        start=(dc == 0),
                        stop=(dc == DC - 1),
                    )
                nc.scalar.activation(
                    out=qkP[:, h, ts(g, 512)],
                    in_=pq,
                    func=AF.Sigmoid,
                    bias=bq_col[:, h : h + 1],
                )

    def s_half(i, h):
        # scores for columns [1024h, 1024h+1024) of row tile i: fp8
        # DoubleRow matmuls (K=256 per instruction) + one fused DVE pass
        # for the rel_pos bias multiply, bf16 cast and row-sum chunk
        sc = sc_ps.tile([128, N // 2], F32, tag="sc", name="sc")
        for c in range(4):
            nc.tensor.matmul(
                sc[:, ts(c, 256)],
                lhsT=qkP[:, :, ts(i, 128)],
                rhs=qkP[:, :, ts(4 * h + c, 256)],
                start=True,
                stop=True,
                perf_mode=DR,
            )
        nc.vector.scalar_tensor_tensor(
            out=P[i][:, ts(h, N // 2)],
            in0=sc,
            scalar=SCALE,
            in1=relf[i][:, ts(h, N // 2)],
            op0=AL.mult,
            op1=AL.mult,
            accum_out=zcs[:, 2 * i + h : 2 * i + h + 1],
        )

    # ---------------- phase A drive: h=0 scores ride the stream ----------
    h0_cursor = 0

    def emit_h0(limit, budget=2):
        nonlocal h0_cursor
        while h0_cursor < min(limit, NT) and budget > 0:
            s_half(h0_cursor, 0)
            h0_cursor += 1
            budget -= 1

    gdone = 0
    for i in range(NT + 2):
        if i < NT:
            a_deg(i)
        if 1 <= i <= NT:
            a_xg(i - 1)
        if i < NT:
            a_gate(i)
        if i >= 2:
            a_qkv(i - 2)
            if (i - 2) % 4 == 3:
                gdone = (i - 2) // 4 + 1
        if gdone >= 2:
            # chunk (t, 0) needs qk groups 0..1 (rhs cols), group t//4
            # (its own lhsT columns) and rel tile t (pacing: t <= i-2)
            emit_h0(min(i - 1, 4 * gdone))
    emit_h0(NT, budget=NT)

    # ---------------- phase B: h=1 scores, transpose, attn @ value --------
    def b1(i):
        s_half(i, 1)
        PT = ptbuf.tile([128, NT, 128], BF16, tag="PT", name="PT")
        nc.sync.dma_start(out=PT, in_=P[i], transpose=True)
        return PT

    def b2(i, PT):
        po = mm_ps.tile([128, 512], F32, tag="mm", name="po")
        for j in range(NT):
            nc.tensor.matmul(
                po[:, 0:D],
                lhsT=PT[:, j, :],
                rhs=vp[j],
                start=(j == 0),
                stop=(j == NT - 1),
            )
        zi = small.tile([128, 1], F32, tag="zi", name="zi")
        nc.vector.tensor_reduce(
            out=zi, in_=zcs[:, 2 * i : 2 * i + 2], axis=mybir.AxisListType.X, op=AL.add
        )
        nc.vector.tensor_scalar_add(out=zi, in0=zi, scalar1=EPS)
        nc.vector.reciprocal(out=zi, in_=zi)
        o = opool.tile([128, D], F32, tag="o", name="o")
        nc.vector.scalar_tensor_tensor(
            out=o, in0=po[:, 0:D], scalar=zi, in1=res[i], op0=AL.mult, op1=AL.add
        )
        nc.scalar.activation(out=o, in_=o, func=AF.Relu)
        nc.scalar.dma_start(out=out_d[ts(i, 128), :], in_=o)

    LAG = 2
    pending = {}
    for i in range(NT + LAG):
        if i >= LAG:
            b2(i - LAG, pending.pop(i - LAG))
        if i < NT:
            pending[i] = b1(i)


_CACHE: dict = {}


def _get_nc():
    if "nc" in _CACHE:
        return _CACHE["nc"], _CACHE["io"]
    nc = bacc.Bacc("TRN2", target_bir_lowering=False, debug=False)
    io = {
        "x": nc.dram_tensor("x", [N, D], F32, kind="ExternalInput").ap(),
        "rel_pos": nc.dram_tensor("rel_pos", [N, N], F32, kind="ExternalInput").ap(),
        "W_qkv": nc.dram_tensor("W_qkv", [E, D], F32, kind="ExternalInput").ap(),
        "b_qkv": nc.dram_tensor("b_qkv", [E], F32, kind="ExternalInput").ap(),
        "W_d": nc.dram_tensor("W_d", [D, 1], F32, kind="ExternalInput").ap(),
        "b_d": nc.dram_tensor("b_d", [D], F32, kind="ExternalInput").ap(),
        "out": nc.dram_tensor("out", [N, D], F32, kind="ExternalOutput").ap(),
    }
    with tile.TileContext(nc) as tc:
        with ExitStack() as ctx:
            build_kernel(ctx, tc, io)
    nc.compile()
    _CACHE["nc"] = nc
    _CACHE["io"] = io
    return nc, io


def kernel(x, rel_pos, W_qkv, b_qkv, W_d, b_d, **run_kwargs):
    nc, _ = _get_nc()
    x = np.ascontiguousarray(np.asarray(x, dtype=np.float32))
    rel_pos = np.ascontiguousarray(np.asarray(rel_pos, dtype=np.float32))
    W_qkv = np.ascontiguousarray(np.asarray(W_qkv, dtype=np.float32))
    b_qkv = np.ascontiguousarray(np.asarray(b_qkv, dtype=np.float32))
    W_d = np.ascontiguousarray(np.asarray(W_d, dtype=np.float32))
    b_d = np.ascontiguousarray(np.asarray(b_d, dtype=np.float32))
    in_maps = [
        {
            "x": x[b],
            "rel_pos": rel_pos[b],
            "W_qkv": W_qkv,
            "b_qkv": b_qkv,
            "W_d": W_d,
            "b_d": b_d,
        }
        for b in range(B)
    ]
    r = run_bass_kernel_spmd(nc, in_maps, core_ids=list(range(B)), **run_kwargs)
    out = np.stack([r.results[b]["out"] for b in range(B)], axis=0)
    if run_kwargs:
        _CACHE["last_result"] = r
    return out


# revision 31
# speedup vs baseline: 1.0210x; 1.0210x over previous
"""Trainium2 Bass kernel for the gated-attention nn.Module.

Math (per batch element b):
    deg   = rel_pos.sum(-1)                        # [N]
    gate  = sigmoid(deg * W_d + b_d)               # [N, D]
    xg    = x * gate
    qkv   = xg @ W_qkv.T + b_qkv                   # [N, 3D]
    qk, value, res = split(qkv); qk = sigmoid(qk)
    attn  = (qk @ qk.T) * scale * rel_pos          # [N, N]
    attn  = attn / (attn.sum(-1, keepdims) + 1e-6)
    out   = relu(attn @ value + res)               # [N, D]

Sharding: pure data-parallel over batch, B == 8 == n_cores, one batch
element per NeuronCore, no collectives.

Per-core schedule (two phases):
  phase A: stream rel_pos f32 on the sync HWDGE queue at full rate and
      keep it resident in f32 (no cast).  Row sums (deg) are computed on
      the DVE for even tiles and on the scalar engine (activation
      accumulator, scratch output) for odd tiles to balance load.
      gate/xg, the XBAR transposes of xg and the qkv projections
      pipeline in the stream's shadow.  qk is written as sigmoid(..) in
      fp8e4 packed [128, 2, N] layout (k-halves on dim 1) for DoubleRow
      matmuls, with its bias folded into the sigmoid activation.
      value is stored fp8 and rearranged into DoubleRow [128, 2, D]
      blocks (partition p holds rows 2p/2p+1 of a 256-row block) with
      parity DMAs on the otherwise idle SWDGE queue.
  phase B: per query row-tile, one DoubleRow fp8 matmul sweep produces
      the full 2048-wide score row in PSUM (K=256 per instruction), a
      single full-row DVE scalar_tensor_tensor fuses the scale+rel_pos
      multiply, the fp8 cast and the row-sum accumulation.  The fp8 row
      is XBAR-transposed as PAIRS via a bf16 bitcast view: the resulting
      layout PT[p, 2q+h] = P[q, 256*j2 + 2p + h] is exactly DoubleRow
      lhsT layout, so attn @ value also runs as 8 DoubleRow matmuls per
      tile (K=256 each) against the interleaved value blocks.  Row
      normalization is applied after the matmul as a per-partition scale
      fused with the residual add.
"""

import math
from contextlib import ExitStack

import numpy as np

import concourse.bass as bass
import concourse.tile as tile
from concourse import bacc, mybir
from concourse.bass import ts
from concourse.bass_utils import run_bass_kernel_spmd
from concourse.masks import make_identity

B, N, D = 8, 2048, 256
E = 3 * D  # 768
NT = N // 128  # 16 row tiles
NB = N // 256  # 8 DoubleRow blocks
DC = D // 128  # 2 dim chunks
SCALE = 1.0 / math.sqrt(32.0)
EPS = 1e-6

F32 = mybir.dt.float32
BF16 = mybir.dt.bfloat16
FP8 = mybir.dt.float8e4

AL = mybir.AluOpType
AF = mybir.ActivationFunctionType
DR = mybir.MatmulPerfMode.DoubleRow
DRSW = mybir.MatmulPerfMode.DoubleRowSwInterleave


def build_kernel(ctx: ExitStack, tc: tile.TileContext, io: dict):
    nc = tc.nc
    x_d = io["x"]          # [N, D]   f32
    rel_d = io["rel_pos"]  # [N, N]   f32
    wq_d = io["W_qkv"]     # [E, D]   f32
    bq_d = io["b_qkv"]     # [E]      f32
    wd_d = io["W_d"]       # [D, 1]   f32
    bd_d = io["b_d"]       # [D]      f32
    out_d = io["out"]      # [N, D]   f32

    # ---------------- pools ----------------
    consts = ctx.enter_context(tc.tile_pool(name="consts", bufs=1))
    resid = ctx.enter_context(tc.tile_pool(name="resid", bufs=1))
    xbuf = ctx.enter_context(tc.tile_pool(name="xbuf", bufs=3))
    ptbuf = ctx.enter_context(tc.tile_pool(name="ptbuf", bufs=3))
    opool = ctx.enter_context(tc.tile_pool(name="opool", bufs=2))
    small = ctx.enter_context(tc.tile_pool(name="small", bufs=8))
    sc_ps = ctx.enter_context(tc.tile_pool(name="sc_ps", bufs=2, space="PSUM"))
    mm_ps = ctx.enter_context(tc.tile_pool(name="mm_ps", bufs=2, space="PSUM"))
    pt_ps = ctx.enter_context(tc.tile_pool(name="pt_ps", bufs=2, space="PSUM"))

    # ---------------- constants ----------------
    ident = consts.tile([128, 128], BF16)
    make_identity(nc, ident)

    wd_bc = consts.tile([128, D], F32)
    nc.sync.dma_start(
        out=wd_bc,
        in_=bass.AP(tensor=wd_d.tensor, offset=wd_d.offset, ap=[[0, 128], [1, D]]),
    )
    bd_bc = consts.tile([128, D], F32)
    nc.sync.dma_start(
        out=bd_bc,
        in_=bass.AP(tensor=bd_d.tensor, offset=bd_d.offset, ap=[[0, 128], [1, D]]),
    )

    ones_row = consts.tile([1, 512], BF16)
    nc.vector.memset(ones_row, 1.0)
    bq_row_f = consts.tile([1, E], F32)
    nc.sync.dma_start(
        out=bq_row_f,
        in_=bass.AP(tensor=bq_d.tensor, offset=bq_d.offset, ap=[[1, 1], [1, E]]),
    )
    bq_row = consts.tile([1, E], BF16)
    nc.vector.tensor_copy(out=bq_row, in_=bq_row_f)
    bq_col = consts.tile([128, DC], F32)
    nc.sync.dma_start(
        out=bq_col,
        in_=bass.AP(tensor=bq_d.tensor, offset=bq_d.offset, ap=[[1, 128], [128, DC]]),
    )

    # W_qkv loaded with SWDGE bf16 cast -> PE-transposed WqT[dc] = W_qkv.T
    wq_nat_bf = consts.tile([128, 6, D], BF16)
    nc.gpsimd.dma_start(out=wq_nat_bf, in_=wq_d.rearrange("(c p) d -> p c d", p=128))
    wqT = [consts.tile([128, E], BF16, tag=f"wqT{dc}", name=f"wqT{dc}") for dc in range(DC)]
    for c in range(6):
        for dc in range(DC):
            pt = pt_ps.tile([128, 128], BF16, tag="pt", name="pt_w", padded_shape=[128, 1024])
            nc.tensor.transpose(pt, wq_nat_bf[:, c, ts(dc, 128)], ident)
            nc.scalar.copy(out=wqT[dc][:, ts(c, 128)], in_=pt)

    # ---------------- resident tensors ----------------
    relf = [resid.tile([128, N], FP8, tag=f"relf{j}", name=f"relf{j}") for j in range(NT)]
    qkP = resid.tile([128, 2, N], FP8)
    xgT = resid.tile([128, 2, N], BF16)
    # value and residual packed per tile: vr[i][:, 0:D] = value (bf16),
    # vr[i][:, D:2D] = res (bf16) — copied from PSUM in ONE activation
    vr = [resid.tile([128, 2 * D], BF16, tag=f"vr{j}", name=f"vr{j}") for j in range(NT)]
    deg = resid.tile([128, NT], F32)
    gates = [resid.tile([128, D], F32, tag=f"gate{j}", name=f"gate{j}") for j in range(NT)]
    # attention score rows, built in column halves as qk chunks land
    P = [resid.tile([128, N], BF16, tag=f"P{j}", name=f"P{j}") for j in range(NT)]
    zcs = resid.tile([128, 2 * NT], F32)

    # x preloaded on the SAME in-order SWDGE queue, AHEAD of the rel
    # stream: guarantees x has landed before the stream monopolizes HBM
    xts = [resid.tile([128, D], F32, tag=f"xt{j}", name=f"xt{j}") for j in range(NT)]
    for i in range(NT):
        nc.gpsimd.dma_start(out=xts[i], in_=x_d[ts(i, 128), :])
    for i in range(NT):
        nc.gpsimd.dma_start(out=relf[i], in_=rel_d[ts(i, 128), :])

    # ---------------- phase A stages ----------------
    def a_deg(i):
        # deg on the scalar engine (activation accumulator, scratch out):
        # DVE is fully booked with gate/xg and the score STTs
        degscr = ptbuf.tile([128, N], BF16, tag="PT", name="degscr")
        nc.scalar.activation(
            out=degscr[:, 0:N].bitcast(FP8)[:, 0:N],
            in_=relf[i],
            func=AF.Copy,
            accum_out=deg[:, i : i + 1],
        )

    def a_gate(i):
        nc.vector.scalar_tensor_tensor(
            out=gates[i],
            in0=wd_bc,
            scalar=deg[:, i : i + 1],
            in1=bd_bc,
            op0=AL.mult,
            op1=AL.add,
        )
        nc.scalar.activation(out=gates[i], in_=gates[i], func=AF.Sigmoid)

    def a_xg(i):
        xg = xbuf.tile([128, D], BF16, tag="xg", name="xg", bufs=6)
        nc.vector.tensor_tensor(out=xg, in0=xts[i], in1=gates[i], op=AL.mult)
        pt = pt_ps.tile([128, 2, 128], BF16, tag="pt", name="pt_xg", padded_shape=[128, 2, 512])
        for dc in range(DC):
            nc.tensor.transpose(pt[:, dc, :], xg[:, ts(dc, 128)], ident)
        nc.scalar.copy(out=xgT[:, :, ts(i, 128)], in_=pt)

    def a_qkv(i):
        pv = mm_ps.tile([128, 512], F32, tag="mm", name="pv")
        for dc in range(DC):
            nc.tensor.matmul(
                pv,
                lhsT=xgT[:, dc, ts(i, 128)],
                rhs=wqT[dc][:, D : 3 * D],
                start=(dc == 0),
                stop=False,
            )
        nc.tensor.matmul(
            pv, lhsT=ones_row[:, 0:128], rhs=bq_row[:, D : 3 * D], start=False, stop=True
        )
        nc.scalar.copy(out=vr[i], in_=pv)
        if i % 4 == 3:
            g = i // 4
            for h in range(DC):
                pq = mm_ps.tile([128, 512], F32, tag="mm", name="pq")
                for dc in range(DC):
                    nc.tensor.matmul(
                        pq,
                        lhsT=wqT[dc][:, ts(h, 128)],
                        rhs=xgT[:, dc, ts(g, 512)],
                        start=(dc == 0),
                        stop=(dc == DC - 1),
                    )
                nc.scalar.activation(
                    out=qkP[:, h, ts(g, 512)],
                    in_=pq,
                    func=AF.Sigmoid,
                    bias=bq_col[:, h : h + 1],
                )

    def s_half(i, h):
        # scores for columns [1024h, 1024h+1024) of row tile i: fp8
        # DoubleRow matmuls (K=256 per instruction) + one fused DVE pass
        # for the rel_pos bias multiply, bf16 cast and row-sum chunk
        sc = sc_ps.tile([128, N // 2], F32, tag="sc", name="sc")
        for c in range(4):
            nc.tensor.matmul(
                sc[:, ts(c, 256)],
                lhsT=qkP[:, :, ts(i, 128)],
                rhs=qkP[:, :, ts(4 * h + c, 256)],
                start=True,
                stop=True,
                perf_mode=DR,
            )
        nc.vector.scalar_tensor_tensor(
            out=P[i][:, ts(h, N // 2)],
            in0=sc,
            scalar=SCALE,
            in1=relf[i][:, ts(h, N // 2)],
            op0=AL.mult,
            op1=AL.mult,
            accum_out=zcs[:, 2 * i + h : 2 * i + h + 1],
        )

    # ---------------- phase A drive: h=0 scores ride the stream ----------
    h0_cursor = 0

    def emit_h0(limit, budget=2):
        nonlocal h0_cursor
        while h0_cursor < min(limit, NT) and budget > 0:
            s_half(h0_cursor, 0)
            h0_cursor += 1
            budget -= 1

    gdone = 0
    for i in range(NT + 2):
        if i < NT:
            a_deg(i)
        if 1 <= i <= NT:
            a_xg(i - 1)
        if i < NT:
            a_gate(i)
        if i >= 2:
            a_qkv(i - 2)
            if (i - 2) % 4 == 3:
                gdone = (i - 2) // 4 + 1
        if gdone >= 2:
            # chunk (t, 0) needs qk groups 0..1 (rhs cols), group t//4
            # (its own lhsT columns) and rel tile t (pacing: t <= i-2)
            emit_h0(min(i - 1, 4 * gdone))
    emit_h0(NT, budget=NT)

    # ---------------- phase B: h=1 scores, transpose, attn @ value --------
    def b1(i):
        s_half(i, 1)
        PT = ptbuf.tile([128, NT, 128], BF16, tag="PT", name="PT")
        nc.sync.dma_start(out=PT, in_=P[i], transpose=True)
        return PT

    def b2(i, PT):
        po = mm_ps.tile([128, 512], F32, tag="mm", name="po")
        for j in range(NT):
            nc.tensor.matmul(
                po[:, 0:D],
                lhsT=PT[:, j, :],
                rhs=vr[j][:, 0:D],
                start=(j == 0),
                stop=(j == NT - 1),
            )
        zi = small.tile([128, 1], F32, tag="zi", name="zi")
        nc.vector.tensor_reduce(
            out=zi, in_=zcs[:, 2 * i : 2 * i + 2], axis=mybir.AxisListType.X, op=AL.add
        )
        nc.vector.tensor_scalar_add(out=zi, in0=zi, scalar1=EPS)
        nc.vector.reciprocal(out=zi, in_=zi)
        o = opool.tile([128, D], F32, tag="o", name="o")
        nc.vector.scalar_tensor_tensor(
            out=o, in0=po[:, 0:D], scalar=zi, in1=vr[i][:, D : 2 * D], op0=AL.mult, op1=AL.add
        )
        nc.scalar.activation(out=o, in_=o, func=AF.Relu)
        nc.scalar.dma_start(out=out_d[ts(i, 128), :], in_=o)

    LAG = 3
    pending = {}
    for i in range(NT + LAG):
        if i >= LAG:
            b2(i - LAG, pending.pop(i - LAG))
        if i < NT:
            pending[i] = b1(i)


_CACHE: dict = {}


def _get_nc():
    if "nc" in _CACHE:
        return _CACHE["nc"], _CACHE["io"]
    nc = bacc.Bacc("TRN2", target_bir_lowering=False, debug=False)
    io = {
        "x": nc.dram_tensor("x", [N, D], F32, kind="ExternalInput").ap(),
        "rel_pos": nc.dram_tensor("rel_pos", [N, N], F32, kind="ExternalInput").ap(),
        "W_qkv": nc.dram_tensor("W_qkv", [E, D], F32, kind="ExternalInput").ap(),
        "b_qkv": nc.dram_tensor("b_qkv", [E], F32, kind="ExternalInput").ap(),
        "W_d": nc.dram_tensor("W_d", [D, 1], F32, kind="ExternalInput").ap(),
        "b_d": nc.dram_tensor("b_d", [D], F32, kind="ExternalInput").ap(),
        "out": nc.dram_tensor("out", [N, D], F32, kind="ExternalOutput").ap(),
    }
    with tile.TileContext(nc) as tc:
        with ExitStack() as ctx:
            build_kernel(ctx, tc, io)
    nc.compile()
    _CACHE["nc"] = nc
    _CACHE["io"] = io
    return nc, io


def kernel(x, rel_pos, W_qkv, b_qkv, W_d, b_d, **run_kwargs):
    nc, _ = _get_nc()
    x = np.ascontiguousarray(np.asarray(x, dtype=np.float32))
    rel_pos = np.ascontiguousarray(np.asarray(rel_pos, dtype=np.float32))
    W_qkv = np.ascontiguousarray(np.asarray(W_qkv, dtype=np.float32))
    b_qkv = np.ascontiguousarray(np.asarray(b_qkv, dtype=np.float32))
    W_d = np.ascontiguousarray(np.asarray(W_d, dtype=np.float32))
    b_d = np.ascontiguousarray(np.asarray(b_d, dtype=np.float32))
    in_maps = [
        {
            "x": x[b],
            "rel_pos": rel_pos[b],
            "W_qkv": W_qkv,
            "b_qkv": b_qkv,
            "W_d": W_d,
            "b_d": b_d,
        }
        for b in range(B)
    ]
    r = run_bass_kernel_spmd(nc, in_maps, core_ids=list(range(B)), **run_kwargs)
    out = np.stack([r.results[b]["out"] for b in range(B)], axis=0)
    if run_kwargs:
        _CACHE["last_result"] = r
    return out
